# revision 15
# baseline (speedup 1.0000x reference)
"""BertMoELayer (B=4, S=2048, H=768, F=3072, E=8, top-2) on 8 Trainium2 cores.

Sharding strategy (per the problem's sharding hint): expert-parallel with
token dispatch by top-k expert index. Expert weights are sharded one expert
per core; the host evaluates the router only to DECIDE the shard assignment
(which tokens go to which core) and gathers each core's token subset, padded
to a static capacity. All numeric computation of the layer runs on device:

  per core c, over its gathered tokens (capacity C):
    logits = x.T @ Wr.T            (fp32 matmul, on device)
    w_c    = top-2 softmax weight of expert c     (on device, fp32)
    hT     = gelu(Wi[c].T^T @ x.T + bi[c])        (bf16 matmul, fp32 psum)
    out_c  = w_c * (hT^T @ Wo[c].T + bo[c])       (bf16 matmul, fp32 psum)

The host unshards by scatter-adding each core's (already weighted) rows:
out[token_list_c] += out_c. Padding rows are dropped (never scattered).

Matmul FLOPs run in bf16 with fp32 accumulation; the router runs entirely in
fp32 so the top-2 selection/weights match the fp32 reference. All tensors are
host-pre-transposed so every DMA is a natural row-major strided load.
"""

import numpy as np
import ml_dtypes

import concourse.bass as bass
import concourse.tile as tile
from concourse import bacc, mybir
from concourse.bass_utils import run_bass_kernel_spmd

B, S, H, F, E = 4, 2048, 768, 3072, 8
T = B * S
N_CORES = 8
TOP_K = 2

P = 128          # SBUF partitions
TB = 512         # token block (matmul free dim)
KH = H // P      # 6   h-chunks
KF = F // P      # 24  f-chunks
HO = 384         # output free-dim split (2 x 384 = 768)

F32 = mybir.dt.float32
BF16 = mybir.dt.bfloat16
BF16_NP = ml_dtypes.bfloat16


def build_nc(cap: int, tb: int = TB):
    """Per-core program: router weight + dense expert FFN over `cap` tokens."""
    assert cap % tb == 0 and tb % P == 0
    ntb = cap // tb
    tsub = tb // P

    # Bacc (not plain Bass): its compile() pass splits multi-wait instructions
    # into event-semaphore chains, which walrus requires (max 1 wait per inst).
    nc = bacc.Bacc(None)

    xgf = nc.declare_dram_parameter("xgf", [H, cap], F32, isOutput=False)
    xg = nc.declare_dram_parameter("xg", [H, cap], BF16, isOutput=False)
    wiT = nc.declare_dram_parameter("wiT", [H, F], BF16, isOutput=False)
    woT = nc.declare_dram_parameter("woT", [F, H], BF16, isOutput=False)
    wrT = nc.declare_dram_parameter("wrT", [H, E], F32, isOutput=False)
    bi = nc.declare_dram_parameter("bi", [F], F32, isOutput=False)
    bo = nc.declare_dram_parameter("bo", [H], F32, isOutput=False)
    esel = nc.declare_dram_parameter("esel", [E], F32, isOutput=False)
    out = nc.declare_dram_parameter("out", [cap, H], F32, isOutput=True)

    # Row-major DRAM views with the contraction dim chunked onto partitions.
    xgf_r = xgf.rearrange("(k p) t -> p k t", p=P)    # [128, KH, cap]
    xg_r = xg.rearrange("(k p) t -> p k t", p=P)      # [128, KH, cap]
    wiT_r = wiT.rearrange("(k p) f -> p k f", p=P)    # [128, KH, F]
    woT_r = woT.rearrange("(j p) f -> p j f", p=P)    # [128, KF, H]
    wrT_r = wrT.rearrange("(k p) e -> p k e", p=P)    # [128, KH, E]
    bi_r = bi.rearrange("(j p) -> p j", p=P)          # [128, KF]
    out_r = out.rearrange("(n p) h -> n p h", p=P)    # [cap/128, 128, H]

    with tile.TileContext(nc) as tc:
        with (
            tc.tile_pool(name="weights", bufs=1) as wpool,
            tc.tile_pool(name="xin", bufs=2) as xpool,
            tc.tile_pool(name="hbuf", bufs=2) as hpool,
            tc.tile_pool(name="obuf", bufs=3) as opool,
            tc.tile_pool(name="router", bufs=2) as rpool,
            tc.tile_pool(name="psum_h", bufs=3, space="PSUM") as ph_pool,
            tc.tile_pool(name="psum_o", bufs=3, space="PSUM") as po_pool,
            tc.tile_pool(name="psum_r", bufs=2, space="PSUM") as pr_pool,
        ):
            # ---- small constants + block-0 tokens first, so the router can
            # start while the big expert-weight DMAs stream in ----
            wrT_sb = wpool.tile([P, KH, E], F32)
            nc.sync.dma_start(out=wrT_sb, in_=wrT_r)
            bi_sb = wpool.tile([P, KF], F32)
            nc.sync.dma_start(out=bi_sb, in_=bi_r)
            # bo broadcast to all 128 partitions (it is added along the free dim)
            bo_sb = wpool.tile([P, H], F32)
            nc.gpsimd.dma_start(out=bo_sb, in_=bo[None, :].to_broadcast([P, H]))
            # one-hot expert selector, broadcast to all partitions
            esel_sb = wpool.tile([P, E], F32)
            nc.gpsimd.dma_start(out=esel_sb, in_=esel[None, :].to_broadcast([P, E]))

            x0_f32 = xpool.tile([P, KH, tb], F32, tag="xf")
            nc.sync.dma_start(out=x0_f32, in_=xgf_r[:, :, 0:tb])
            x0_bf = xpool.tile([P, KH, tb], BF16, tag="xb")
            nc.sync.dma_start(out=x0_bf, in_=xg_r[:, :, 0:tb])

            # ---- big expert weights ----
            wiT_sb = wpool.tile([P, KH, F], BF16)
            nc.sync.dma_start(out=wiT_sb, in_=wiT_r)
            woT_sb = wpool.tile([P, KF, H], BF16)
            nc.sync.dma_start(out=woT_sb, in_=woT_r)

            for tbi in range(ntb):
                t0 = tbi * tb
                if tbi == 0:
                    x_f32, x_bf = x0_f32, x0_bf
                else:
                    x_f32 = xpool.tile([P, KH, tb], F32, tag="xf")
                    nc.sync.dma_start(out=x_f32, in_=xgf_r[:, :, t0 : t0 + tb])
                    x_bf = xpool.tile([P, KH, tb], BF16, tag="xb")
                    nc.sync.dma_start(out=x_bf, in_=xg_r[:, :, t0 : t0 + tb])

                # ---- router: logits -> top-2 softmax -> this expert's weight ----
                w_blk = rpool.tile([P, tsub], F32, tag="w")
                for ts in range(tsub):
                    psl = pr_pool.tile([P, E], F32, tag="pr")
                    for k in range(KH):
                        nc.tensor.matmul(
                            psl,
                            lhsT=x_f32[:, k, ts * P : (ts + 1) * P],
                            rhs=wrT_sb[:, k, :],
                            start=(k == 0),
                            stop=(k == KH - 1),
                        )
                    lg = rpool.tile([P, E], F32, tag="lg")
                    nc.vector.tensor_copy(out=lg, in_=psl)
                    m1 = rpool.tile([P, 1], F32, tag="m1")
                    nc.vector.reduce_max(m1, lg, axis=mybir.AxisListType.X)
                    # mask out the argmax, then find the 2nd max
                    ge = rpool.tile([P, E], F32, tag="ge")
                    nc.vector.tensor_scalar(
                        ge, lg, scalar1=m1, scalar2=-1e30,
                        op0=mybir.AluOpType.is_ge, op1=mybir.AluOpType.mult,
                    )
                    mk = rpool.tile([P, E], F32, tag="mk")
                    nc.vector.tensor_tensor(mk, lg, ge, op=mybir.AluOpType.add)
                    m2 = rpool.tile([P, 1], F32, tag="m2")
                    nc.vector.reduce_max(m2, mk, axis=mybir.AxisListType.X)
                    # softmax over {m1, m2}: w1 = 1/(1+e2), w2 = e2/(1+e2)
                    dd = rpool.tile([P, 1], F32, tag="dd")
                    nc.vector.tensor_tensor(dd, m2, m1, op=mybir.AluOpType.subtract)
                    e2 = rpool.tile([P, 1], F32, tag="e2")
                    nc.scalar.activation(e2, dd, mybir.ActivationFunctionType.Exp)
                    den = rpool.tile([P, 1], F32, tag="den")
                    nc.vector.tensor_scalar_add(den, e2, 1.0)
                    w1 = rpool.tile([P, 1], F32, tag="w1")
                    nc.vector.reciprocal(w1, den)
                    w2 = rpool.tile([P, 1], F32, tag="w2")
                    nc.vector.tensor_tensor(w2, e2, w1, op=mybir.AluOpType.mult)
                    # this core's logit: lc = sum(lg * esel)
                    lc_t = rpool.tile([P, E], F32, tag="lct")
                    nc.vector.tensor_tensor(lc_t, lg, esel_sb, op=mybir.AluOpType.mult)
                    lc = rpool.tile([P, 1], F32, tag="lc")
                    nc.vector.reduce_sum(lc, lc_t, axis=mybir.AxisListType.X)
                    d1 = rpool.tile([P, 1], F32, tag="d1")
                    nc.vector.tensor_scalar(
                        d1, lc, scalar1=m1, scalar2=None, op0=mybir.AluOpType.is_ge
                    )
                    g2 = rpool.tile([P, 1], F32, tag="g2")
                    nc.vector.tensor_scalar(
                        g2, lc, scalar1=m2, scalar2=None, op0=mybir.AluOpType.is_ge
                    )
                    # w = d1*(w1-w2) + g2*w2
                    wa = rpool.tile([P, 1], F32, tag="wa")
                    nc.vector.tensor_tensor(wa, w1, w2, op=mybir.AluOpType.subtract)
                    t1 = rpool.tile([P, 1], F32, tag="t1")
                    nc.vector.tensor_tensor(t1, d1, wa, op=mybir.AluOpType.mult)
                    t2 = rpool.tile([P, 1], F32, tag="t2")
                    nc.vector.tensor_tensor(t2, g2, w2, op=mybir.AluOpType.mult)
                    nc.vector.tensor_tensor(
                        w_blk[:, ts : ts + 1], t1, t2, op=mybir.AluOpType.add
                    )

                # ---- expert FFN layer 1: hT[f, t] = gelu(WiT^T @ xT + bi) ----
                hT = hpool.tile([P, KF, tb], BF16, tag="hT")
                for j in range(KF):
                    ps = ph_pool.tile([P, tb], F32, tag="ph")
                    for k in range(KH):
                        nc.tensor.matmul(
                            ps,
                            lhsT=wiT_sb[:, k, j * P : (j + 1) * P],
                            rhs=x_bf[:, k, :],
                            start=(k == 0),
                            stop=(k == KH - 1),
                        )
                    nc.scalar.activation(
                        out=hT[:, j, :],
                        in_=ps,
                        func=mybir.ActivationFunctionType.Gelu,
                        bias=bi_sb[:, j : j + 1],
                        scale=1.0,
                    )

                # ---- layer 2 + bo + routing-weight scale ----
                for ts in range(tsub):
                    po_a = po_pool.tile([P, HO], F32, tag="po")
                    po_b = po_pool.tile([P, HO], F32, tag="po")
                    for j in range(KF):
                        lhsT = hT[:, j, ts * P : (ts + 1) * P]
                        nc.tensor.matmul(
                            po_a, lhsT=lhsT, rhs=woT_sb[:, j, 0:HO],
                            start=(j == 0), stop=(j == KF - 1),
                        )
                        nc.tensor.matmul(
                            po_b, lhsT=lhsT, rhs=woT_sb[:, j, HO : 2 * HO],
                            start=(j == 0), stop=(j == KF - 1),
                        )
                    o_sb = opool.tile([P, H], F32, tag="os")
                    wcol = w_blk[:, ts : ts + 1]
                    nc.vector.tensor_tensor(
                        o_sb[:, 0:HO], po_a, bo_sb[:, 0:HO], op=mybir.AluOpType.add
                    )
                    nc.vector.tensor_tensor(
                        o_sb[:, HO : 2 * HO], po_b, bo_sb[:, HO : 2 * HO],
                        op=mybir.AluOpType.add,
                    )
                    nc.vector.tensor_scalar_mul(o_sb, o_sb, scalar1=wcol)
                    nc.sync.dma_start(out=out_r[tbi * tsub + ts], in_=o_sb)

    nc.compile()
    return nc


_NC_CACHE: dict = {}


def _get_nc(cap: int):
    if cap not in _NC_CACHE:
        _NC_CACHE[cap] = build_nc(cap)
    return _NC_CACHE[cap]


def _shard_tokens(xf, Wr):
    """Host-side sharding function: top-2 expert index per token (matches
    jax.lax.top_k tie-breaking: lowest index wins on ties)."""
    logits = xf.astype(np.float32) @ np.asarray(Wr, np.float32).T  # [T, E]
    i1 = np.argmax(logits, axis=1)
    l2 = logits.copy()
    l2[np.arange(len(i1)), i1] = -np.inf
    i2 = np.argmax(l2, axis=1)
    tokens = np.arange(logits.shape[0])
    tok_lists = []
    for c in range(N_CORES):
        tok_lists.append(np.concatenate([tokens[i1 == c], tokens[i2 == c]]))
    return tok_lists


def kernel(x, Wr, Wi, bi, Wo, bo, _trace=False):
    x = np.asarray(x)
    xf = x.reshape(-1, H).astype(np.float32)
    tok_lists = _shard_tokens(xf, Wr)
    maxc = max(len(tl) for tl in tok_lists)
    cap = max(TB, int(np.ceil(maxc / TB) * TB))

    xT = np.ascontiguousarray(xf.T)  # [H, T] fp32
    in_maps = []
    for c in range(N_CORES):
        tl = tok_lists[c]
        xgf = np.zeros((H, cap), dtype=np.float32)
        xgf[:, : len(tl)] = xT[:, tl]
        sel = np.zeros(E, np.float32)
        sel[c] = 1.0
        in_maps.append(
            {
                "xgf": xgf,
                "xg": xgf.astype(BF16_NP),
                "wiT": np.ascontiguousarray(np.asarray(Wi[c], np.float32).T).astype(
                    BF16_NP
                ),
                "woT": np.ascontiguousarray(np.asarray(Wo[c], np.float32).T).astype(
                    BF16_NP
                ),
                "wrT": np.ascontiguousarray(np.asarray(Wr, np.float32).T),
                "bi": np.asarray(bi[c], np.float32),
                "bo": np.asarray(bo[c], np.float32),
                "esel": sel,
            }
        )

    nc = _get_nc(cap)
    res = run_bass_kernel_spmd(
        nc, in_maps, core_ids=list(range(N_CORES)), trace=_trace
    )

    # Unshard: scatter-add the per-expert (already routing-weighted) rows.
    out = np.zeros((T, H), dtype=np.float32)
    for c in range(N_CORES):
        tl = tok_lists[c]
        out[tl] += res.results[c]["out"][: len(tl)]
    out = out.reshape(x.shape)
    if _trace:
        return out, res
    return out


# revision 18
# speedup vs baseline: 1.0285x; 1.0285x over previous
"""BertMoELayer (B=4, S=2048, H=768, F=3072, E=8, top-2) on 8 Trainium2 cores.

Sharding strategy (per the problem's sharding hint): expert-parallel with
token dispatch by top-k expert index. Expert weights are sharded one expert
per core; the host evaluates the router only to DECIDE the shard assignment
(which tokens go to which core) and gathers each core's token subset, padded
to a static capacity. All numeric computation of the layer runs on device:

  per core c, over its gathered tokens (capacity C):
    logits = x.T @ Wr.T            (fp32 matmul, on device)
    w_c    = top-2 softmax weight of expert c     (on device, fp32)
    hT     = gelu(Wi[c].T^T @ x.T + bi[c])        (bf16 matmul, fp32 psum)
    out_c  = w_c * (hT^T @ Wo[c].T + bo[c])       (bf16 matmul, fp32 psum)

The host unshards by scatter-adding each core's (already weighted) rows:
out[token_list_c] += out_c. Padding rows are dropped (never scattered).

Matmul FLOPs run in bf16 with fp32 accumulation; the router runs entirely in
fp32 so the top-2 selection/weights match the fp32 reference. All tensors are
host-pre-transposed so every DMA is a natural row-major strided load.
"""

import numpy as np
import ml_dtypes

import concourse.bass as bass
import concourse.tile as tile
from concourse import bacc, mybir
from concourse.bass_utils import run_bass_kernel_spmd

B, S, H, F, E = 4, 2048, 768, 3072, 8
T = B * S
N_CORES = 8
TOP_K = 2

P = 128          # SBUF partitions
TB = 512         # token block (matmul free dim)
KH = H // P      # 6   h-chunks
KF = F // P      # 24  f-chunks
HO = 384         # output free-dim split (2 x 384 = 768)

F32 = mybir.dt.float32
BF16 = mybir.dt.bfloat16
BF16_NP = ml_dtypes.bfloat16


def build_nc(cap: int, tb: int = TB):
    """Per-core program: router weight + dense expert FFN over `cap` tokens."""
    assert cap % tb == 0 and tb % P == 0
    ntb = cap // tb
    tsub = tb // P

    # Bacc (not plain Bass): its compile() pass splits multi-wait instructions
    # into event-semaphore chains, which walrus requires (max 1 wait per inst).
    nc = bacc.Bacc(None)

    xgf = nc.declare_dram_parameter("xgf", [H, cap], F32, isOutput=False)
    xg = nc.declare_dram_parameter("xg", [H, cap], BF16, isOutput=False)
    wiT = nc.declare_dram_parameter("wiT", [H, F], BF16, isOutput=False)
    woT = nc.declare_dram_parameter("woT", [F, H], BF16, isOutput=False)
    wrT = nc.declare_dram_parameter("wrT", [H, E], F32, isOutput=False)
    bi = nc.declare_dram_parameter("bi", [F], F32, isOutput=False)
    bo = nc.declare_dram_parameter("bo", [H], F32, isOutput=False)
    esel = nc.declare_dram_parameter("esel", [E], F32, isOutput=False)
    out = nc.declare_dram_parameter("out", [cap, H], F32, isOutput=True)

    # Row-major DRAM views with the contraction dim chunked onto partitions.
    xgf_r = xgf.rearrange("(k p) t -> p k t", p=P)    # [128, KH, cap]
    xg_r = xg.rearrange("(k p) t -> p k t", p=P)      # [128, KH, cap]
    wiT_r = wiT.rearrange("(k p) f -> p k f", p=P)    # [128, KH, F]
    woT_r = woT.rearrange("(j p) f -> p j f", p=P)    # [128, KF, H]
    wrT_r = wrT.rearrange("(k p) e -> p k e", p=P)    # [128, KH, E]
    bi_r = bi.rearrange("(j p) -> p j", p=P)          # [128, KF]
    out_r = out.rearrange("(n p) h -> n p h", p=P)    # [cap/128, 128, H]

    with tile.TileContext(nc) as tc:
        with (
            tc.tile_pool(name="weights", bufs=1) as wpool,
            tc.tile_pool(name="xin", bufs=2) as xpool,
            tc.tile_pool(name="hbuf", bufs=2) as hpool,
            tc.tile_pool(name="obuf", bufs=3) as opool,
            tc.tile_pool(name="router", bufs=2) as rpool,
            tc.tile_pool(name="psum_h", bufs=3, space="PSUM") as ph_pool,
            tc.tile_pool(name="psum_o", bufs=3, space="PSUM") as po_pool,
            tc.tile_pool(name="psum_r", bufs=2, space="PSUM") as pr_pool,
        ):
            # ---- persistent weights / constants ----
            wiT_sb = wpool.tile([P, KH, F], BF16)
            nc.sync.dma_start(out=wiT_sb, in_=wiT_r)
            woT_sb = wpool.tile([P, KF, H], BF16)
            nc.sync.dma_start(out=woT_sb, in_=woT_r)
            wrT_sb = wpool.tile([P, KH, E], F32)
            nc.sync.dma_start(out=wrT_sb, in_=wrT_r)
            bi_sb = wpool.tile([P, KF], F32)
            nc.sync.dma_start(out=bi_sb, in_=bi_r)
            # bo broadcast to all 128 partitions (it is added along the free dim)
            bo_sb = wpool.tile([P, H], F32)
            nc.gpsimd.dma_start(out=bo_sb, in_=bo[None, :].to_broadcast([P, H]))
            # one-hot expert selector, broadcast to all partitions
            esel_sb = wpool.tile([P, E], F32)
            nc.gpsimd.dma_start(out=esel_sb, in_=esel[None, :].to_broadcast([P, E]))

            for tbi in range(ntb):
                t0 = tbi * tb
                x_f32 = xpool.tile([P, KH, tb], F32, tag="xf")
                nc.sync.dma_start(out=x_f32, in_=xgf_r[:, :, t0 : t0 + tb])
                x_bf = xpool.tile([P, KH, tb], BF16, tag="xb")
                nc.sync.dma_start(out=x_bf, in_=xg_r[:, :, t0 : t0 + tb])

                # ---- router: logits -> top-2 softmax -> this expert's weight ----
                w_blk = rpool.tile([P, tsub], F32, tag="w")
                for ts in range(tsub):
                    psl = pr_pool.tile([P, E], F32, tag="pr")
                    for k in range(KH):
                        nc.tensor.matmul(
                            psl,
                            lhsT=x_f32[:, k, ts * P : (ts + 1) * P],
                            rhs=wrT_sb[:, k, :],
                            start=(k == 0),
                            stop=(k == KH - 1),
                        )
                    lg = rpool.tile([P, E], F32, tag="lg")
                    nc.vector.tensor_copy(out=lg, in_=psl)
                    m1 = rpool.tile([P, 1], F32, tag="m1")
                    nc.vector.reduce_max(m1, lg, axis=mybir.AxisListType.X)
                    # mask out the argmax, then find the 2nd max
                    ge = rpool.tile([P, E], F32, tag="ge")
                    nc.vector.tensor_scalar(
                        ge, lg, scalar1=m1, scalar2=-1e30,
                        op0=mybir.AluOpType.is_ge, op1=mybir.AluOpType.mult,
                    )
                    mk = rpool.tile([P, E], F32, tag="mk")
                    nc.vector.tensor_tensor(mk, lg, ge, op=mybir.AluOpType.add)
                    m2 = rpool.tile([P, 1], F32, tag="m2")
                    nc.vector.reduce_max(m2, mk, axis=mybir.AxisListType.X)
                    # softmax over {m1, m2}: w1 = 1/(1+e2), w2 = e2/(1+e2)
                    dd = rpool.tile([P, 1], F32, tag="dd")
                    nc.vector.tensor_tensor(dd, m2, m1, op=mybir.AluOpType.subtract)
                    e2 = rpool.tile([P, 1], F32, tag="e2")
                    nc.scalar.activation(e2, dd, mybir.ActivationFunctionType.Exp)
                    den = rpool.tile([P, 1], F32, tag="den")
                    nc.vector.tensor_scalar_add(den, e2, 1.0)
                    w1 = rpool.tile([P, 1], F32, tag="w1")
                    nc.vector.reciprocal(w1, den)
                    w2 = rpool.tile([P, 1], F32, tag="w2")
                    nc.vector.tensor_tensor(w2, e2, w1, op=mybir.AluOpType.mult)
                    # this core's logit: lc = sum(lg * esel)
                    lc_t = rpool.tile([P, E], F32, tag="lct")
                    nc.vector.tensor_tensor(lc_t, lg, esel_sb, op=mybir.AluOpType.mult)
                    lc = rpool.tile([P, 1], F32, tag="lc")
                    nc.vector.reduce_sum(lc, lc_t, axis=mybir.AxisListType.X)
                    d1 = rpool.tile([P, 1], F32, tag="d1")
                    nc.vector.tensor_scalar(
                        d1, lc, scalar1=m1, scalar2=None, op0=mybir.AluOpType.is_ge
                    )
                    g2 = rpool.tile([P, 1], F32, tag="g2")
                    nc.vector.tensor_scalar(
                        g2, lc, scalar1=m2, scalar2=None, op0=mybir.AluOpType.is_ge
                    )
                    # w = d1*(w1-w2) + g2*w2
                    wa = rpool.tile([P, 1], F32, tag="wa")
                    nc.vector.tensor_tensor(wa, w1, w2, op=mybir.AluOpType.subtract)
                    t1 = rpool.tile([P, 1], F32, tag="t1")
                    nc.vector.tensor_tensor(t1, d1, wa, op=mybir.AluOpType.mult)
                    t2 = rpool.tile([P, 1], F32, tag="t2")
                    nc.vector.tensor_tensor(t2, g2, w2, op=mybir.AluOpType.mult)
                    nc.vector.tensor_tensor(
                        w_blk[:, ts : ts + 1], t1, t2, op=mybir.AluOpType.add
                    )

                # ---- expert FFN layer 1: hT[f, t] = gelu(WiT^T @ xT + bi) ----
                hT = hpool.tile([P, KF, tb], BF16, tag="hT")
                for j in range(KF):
                    ps = ph_pool.tile([P, tb], F32, tag="ph")
                    for k in range(KH):
                        nc.tensor.matmul(
                            ps,
                            lhsT=wiT_sb[:, k, j * P : (j + 1) * P],
                            rhs=x_bf[:, k, :],
                            start=(k == 0),
                            stop=(k == KH - 1),
                        )
                    nc.scalar.activation(
                        out=hT[:, j, :],
                        in_=ps,
                        func=mybir.ActivationFunctionType.Gelu,
                        bias=bi_sb[:, j : j + 1],
                        scale=1.0,
                    )

                # ---- layer 2 + bo + routing-weight scale ----
                for ts in range(tsub):
                    po_a = po_pool.tile([P, HO], F32, tag="po")
                    po_b = po_pool.tile([P, HO], F32, tag="po")
                    for j in range(KF):
                        lhsT = hT[:, j, ts * P : (ts + 1) * P]
                        nc.tensor.matmul(
                            po_a, lhsT=lhsT, rhs=woT_sb[:, j, 0:HO],
                            start=(j == 0), stop=(j == KF - 1),
                        )
                        nc.tensor.matmul(
                            po_b, lhsT=lhsT, rhs=woT_sb[:, j, HO : 2 * HO],
                            start=(j == 0), stop=(j == KF - 1),
                        )
                    o_sb = opool.tile([P, H], F32, tag="os")
                    wcol = w_blk[:, ts : ts + 1]
                    nc.vector.tensor_tensor(
                        o_sb[:, 0:HO], po_a, bo_sb[:, 0:HO], op=mybir.AluOpType.add
                    )
                    nc.vector.tensor_tensor(
                        o_sb[:, HO : 2 * HO], po_b, bo_sb[:, HO : 2 * HO],
                        op=mybir.AluOpType.add,
                    )
                    nc.vector.tensor_scalar_mul(o_sb, o_sb, scalar1=wcol)
                    nc.sync.dma_start(out=out_r[tbi * tsub + ts], in_=o_sb)

    nc.compile()
    return nc


_NC_CACHE: dict = {}


def _get_nc(cap: int):
    if cap not in _NC_CACHE:
        _NC_CACHE[cap] = build_nc(cap)
    return _NC_CACHE[cap]


def _ensure_axon_hooks_module():
    """run_bass_kernel_spmd(trace=True) (e.g. via env BASS_TRACE=1) imports
    antenv.axon_hooks, which some images lack even though the boot code that
    would register the NTFF hook is present. Provide the module and register
    the real hook when available so tracing works instead of crashing."""
    try:
        import antenv.axon_hooks  # noqa: F401

        return
    except ImportError:
        pass
    try:
        import sys
        import types

        import antenv  # noqa: F401

        mod = types.ModuleType("antenv.axon_hooks")
        state = {"hook": None}
        mod.set_axon_ntff_profile_hook = lambda h: state.__setitem__("hook", h)
        mod.get_axon_ntff_profile_hook = lambda: state["hook"]
        try:
            from trn_agent_boot.trn_boot import _ntff_profile_via_ctypes

            mod.set_axon_ntff_profile_hook(
                _ntff_profile_via_ctypes("/opt/axon/libaxon_pjrt.so")
            )
        except Exception:
            pass
        sys.modules["antenv.axon_hooks"] = mod
    except Exception:
        pass


def _shard_tokens(xf, Wr):
    """Host-side sharding function: top-2 expert index per token (matches
    jax.lax.top_k tie-breaking: lowest index wins on ties)."""
    logits = xf.astype(np.float32) @ np.asarray(Wr, np.float32).T  # [T, E]
    i1 = np.argmax(logits, axis=1)
    l2 = logits.copy()
    l2[np.arange(len(i1)), i1] = -np.inf
    i2 = np.argmax(l2, axis=1)
    tokens = np.arange(logits.shape[0])
    tok_lists = []
    for c in range(N_CORES):
        tok_lists.append(np.concatenate([tokens[i1 == c], tokens[i2 == c]]))
    return tok_lists


def kernel(x, Wr, Wi, bi, Wo, bo, _trace=False):
    x = np.asarray(x)
    xf = x.reshape(-1, H).astype(np.float32)
    tok_lists = _shard_tokens(xf, Wr)
    maxc = max(len(tl) for tl in tok_lists)
    cap = max(TB, int(np.ceil(maxc / TB) * TB))

    xT = np.ascontiguousarray(xf.T)  # [H, T] fp32
    in_maps = []
    for c in range(N_CORES):
        tl = tok_lists[c]
        xgf = np.zeros((H, cap), dtype=np.float32)
        xgf[:, : len(tl)] = xT[:, tl]
        sel = np.zeros(E, np.float32)
        sel[c] = 1.0
        in_maps.append(
            {
                "xgf": xgf,
                "xg": xgf.astype(BF16_NP),
                "wiT": np.ascontiguousarray(np.asarray(Wi[c], np.float32).T).astype(
                    BF16_NP
                ),
                "woT": np.ascontiguousarray(np.asarray(Wo[c], np.float32).T).astype(
                    BF16_NP
                ),
                "wrT": np.ascontiguousarray(np.asarray(Wr, np.float32).T),
                "bi": np.asarray(bi[c], np.float32),
                "bo": np.asarray(bo[c], np.float32),
                "esel": sel,
            }
        )

    _ensure_axon_hooks_module()
    nc = _get_nc(cap)
    res = run_bass_kernel_spmd(
        nc, in_maps, core_ids=list(range(N_CORES)), trace=_trace
    )

    # Unshard: scatter-add the per-expert (already routing-weighted) rows.
    out = np.zeros((T, H), dtype=np.float32)
    for c in range(N_CORES):
        tl = tok_lists[c]
        out[tl] += res.results[c]["out"][: len(tl)]
    out = out.reshape(x.shape)
    if _trace:
        return out, res
    return out
